# revision 20
# baseline (speedup 1.0000x reference)
"""Bass/Trainium2 kernel for nn_LocalAggregator (GNN message passing).

Math per batch b (hidden [64,128], adj [64,64] in {0..4}, a [4,128]):
    e_k[i,j] = leakyrelu_{0.2}( sum_d hidden[i,d]*hidden[j,d]*a[k,d] )
    alpha    = softmax_j( where(adj==k+1, e_k, -9e15) )
    out      = alpha @ hidden

Device strategy (8 cores, 64 batches/core). Matmuls + PSUM run per
QUAD (4 batches) for deep pipelining; elementwise runs per OCT
(8 batches) to amortize per-op fixed costs:
  - e_k is SYMMETRIC in (i,j): masking with the host-TRANSPOSED
    adjacency yields transposed attention weights directly.
  - w_all[d,(k,l,j)] = hT * a_k precomputed on HOST, shipped in the
    single fused oct DMA (one dma_start per 8 batches).
  - e-matmuls write a STRIDED PSUM AP -> e4 cols are (k,t,j); Prelu
    scatters quad halves into a (k, g', t, c) oct tile so Exp /
    one-hot eq / mask-mul / k-sum adds are contiguous full-width ops.
  - ones-column in hh makes the out-matmul emit the softmax
    denominator; normalization on HOST; f32->bf16 cast inside the
    SWDGE output DMA on the otherwise idle GpSimd engine.
"""

import numpy as np
import ml_dtypes

from contextlib import ExitStack

import concourse.bass as bass
import concourse.tile as tile
from concourse import bacc, mybir
from concourse._compat import with_exitstack
from concourse.bass_utils import run_bass_kernel_spmd

BF16 = mybir.dt.bfloat16
F32 = mybir.dt.float32
ALU = mybir.AluOpType
ACTF = mybir.ActivationFunctionType

B, N, D, K = 512, 64, 128, 4
NCORES = 8
BPC = B // NCORES          # 64 batches per core
QUADS = BPC // 4           # 16 quads of 4 batches per core
OCTS = BPC // 8            # 8 octs of 8 batches per core
HHW = 132                  # hidden cols + ones col + pad
# fused oct input cols: A = hT8 | wall8 ; B = adj8 | hh8
CWA = 512 + 2048                         # = 2560
CWB = 256 + 4 * HHW                      # = 784
OW = 2 * HHW               # out tile cols: (num 128 | denom | pad) x 2


@with_exitstack
def _kernel_body(ctx, tc, ina_d, inb_d, out_d):
    nc = tc.nc

    ina_pool = ctx.enter_context(tc.tile_pool(name="inpa", bufs=6))
    inb_pool = ctx.enter_context(tc.tile_pool(name="inpb", bufs=8))
    work_pool = ctx.enter_context(tc.tile_pool(name="work", bufs=8))
    psum_pool = ctx.enter_context(tc.tile_pool(name="psum", bufs=4, space="PSUM"))
    opsum_pool = ctx.enter_context(tc.tile_pool(name="opsum", bufs=2, space="PSUM"))
    out_pool = ctx.enter_context(tc.tile_pool(name="outp", bufs=8))

    for g in range(OCTS):
        # ---- two fused oct loads ----
        # A (released after e-matmuls):
        #   0:512     hT8   [128=d, (g',l,i)]     hidden^T, 8 batches
        #   512:2560  wall8 [128=d, (g',k,l,j)]   hT * a_k
        # B (small; held to the out-matmuls):
        #   0:256     adj8  [128=(u,r), (g',t,c)] transposed adjacency
        #   256:784   hh8   [128=(u,j), (g',t,c)] hidden rows + ones col
        cmbb = inb_pool.tile([128, CWB], BF16, tag="cmbb")
        nc.sync.dma_start(out=cmbb[:], in_=inb_d[g])
        cmb = ina_pool.tile([128, CWA], BF16, tag="cmba")
        nc.sync.dma_start(out=cmb[:], in_=ina_d[g])
        adj8 = cmbb[:, 0:256]
        wall8v = cmb[:, 512:CWA].rearrange(
            "p (g k l j) -> p g k l j", g=2, k=4, l=4)

        # one-hot indicators depend only on the small B load: run them
        # early so DVE works during the matmul/Prelu window
        ind8 = work_pool.tile([128, 1024], BF16, tag="ind8")
        for k in range(K):
            nc.vector.tensor_scalar(
                ind8[:, k * 256: (k + 1) * 256], adj8, float(k + 1),
                None, ALU.is_equal)

        # oct-wide elementwise tiles, col layout (k, g', t, c)
        lr8 = work_pool.tile([128, 1024], F32, tag="lr8")
        lr8v = lr8[:].rearrange("p (k g tc) -> p k g tc", k=4, g=2)

        e4s = []
        for gp in range(2):
            # ---- e4[(u,i), (k,t,j)] : 4 matmuls (strided PSUM out) ----
            e4 = psum_pool.tile([128, 512], F32, tag="e4")
            e4v = e4[:].rearrange("p (k t j) -> p k t j", k=4, t=2)
            for l in range(4):
                t, u = l // 2, l % 2
                nc.tensor.matmul(
                    e4v[u * 64: (u + 1) * 64, :, t, :],
                    lhsT=cmb[:, gp * 256 + l * 64: gp * 256 + (l + 1) * 64],
                    rhs=wall8v[:, gp, :, l, :],
                    start=True, stop=True,
                    tile_position=(0, u * 64),
                )
            e4s.append(e4)
            # ---- leakyrelu evacuates PSUM into the oct tile ----
            nc.scalar.activation(
                lr8v[:, :, gp, :],
                e4[:].rearrange("p (k tc) -> p k tc", k=4),
                ACTF.Prelu, alpha=0.2)

        # ---- oct-wide: exp, one-hot select, k-sum ----
        xm8 = work_pool.tile([128, 1024], BF16, tag="xm8")
        nc.scalar.activation(xm8[:], lr8[:], ACTF.Exp)
        w8 = work_pool.tile([128, 1024], BF16, tag="w8")
        nc.vector.tensor_mul(w8[:], xm8[:], ind8[:])
        t2 = work_pool.tile([128, 512], BF16, tag="t2")
        nc.vector.tensor_tensor(t2[:], w8[:, 0:512], w8[:, 512:1024], ALU.add)
        wsum = work_pool.tile([128, 256], BF16, tag="wsum")
        nc.vector.tensor_tensor(wsum[:], t2[:, 0:256], t2[:, 256:512], ALU.add)

        # ---- out matmuls: ONE 2-bank PSUM tile per oct ----
        # (gp halves sit at the 512-col bank boundary: no MM crosses a bank)
        ops = opsum_pool.tile([128, 1024], F32, tag="ops")
        for gp in range(2):
            for l in range(4):
                t, u = l // 2, l % 2
                nc.tensor.matmul(
                    ops[u * 64: (u + 1) * 64,
                        gp * 512 + t * HHW: gp * 512 + (t + 1) * HHW],
                    lhsT=wsum[u * 64: (u + 1) * 64,
                              gp * 128 + t * 64: gp * 128 + (t + 1) * 64],
                    rhs=cmbb[u * 64: (u + 1) * 64,
                             256 + (gp * 2 + t) * HHW: 256 + (gp * 2 + t + 1) * HHW],
                    start=True, stop=True,
                    tile_position=(u * 64, u * 64),
                )
        # ---- single compact evac + single output DMA per oct ----
        osb = out_pool.tile([128, 2 * OW], BF16, tag="osb")
        nc.vector.tensor_copy(
            osb[:].rearrange("p (g c) -> p g c", g=2),
            ops[:].rearrange("p (g c) -> p g c", g=2)[:, :, 0:OW])
        nc.gpsimd.dma_start(out=out_d[g], in_=osb[:])


def build_nc():
    nc = bacc.Bacc("TRN2", target_bir_lowering=False, debug=False)
    ina_d = nc.dram_tensor("cmba", [OCTS, 128, CWA], BF16,
                           kind="ExternalInput").ap()
    inb_d = nc.dram_tensor("cmbb", [OCTS, 128, CWB], BF16,
                           kind="ExternalInput").ap()
    out_d = nc.dram_tensor("out", [OCTS, 128, 2 * OW], BF16,
                           kind="ExternalOutput").ap()
    with tile.TileContext(nc) as tc:
        _kernel_body(tc, ina_d, inb_d, out_d)
    nc.compile()
    return nc


def _octify(x):
    """[B//4, 128, W] -> [B//8, 128, 2*W] pairing consecutive quads."""
    q, p, w = x.shape
    return (x.reshape(q // 2, 2, p, w).transpose(0, 2, 1, 3)
            .reshape(q // 2, p, 2 * w))


def prep_inputs(hidden, adj, a):
    """Host-side packing: bf16 casts, fused transposed/interleaved layouts."""
    bf = ml_dtypes.bfloat16
    hidden = np.asarray(hidden, dtype=np.float32)
    adj = np.asarray(adj)
    a = np.asarray(a, dtype=np.float32)

    hb = hidden.astype(bf)                                   # [B, 64, 128]

    # hT_q[q, d, l*64+i] = hidden[4q+l, i, d]
    hTf = (hidden.transpose(0, 2, 1)
           .reshape(B // 4, 4, D, N)
           .transpose(0, 2, 1, 3)
           .reshape(B // 4, D, 4 * N))
    hT = hTf.astype(bf)

    # adjT_q[q, u*64+r, t*64+c] = adj[4q+2t+u][c, r]
    adjT = adj.transpose(0, 2, 1).astype(bf)
    adjTq = (adjT.reshape(B // 4, 2, 2, N, N)
             .transpose(0, 2, 3, 1, 4)
             .reshape(B // 4, 2 * N, 2 * N))

    # hh_q[q, u*64+j, t*HHW + c] : hidden rows + ones col for batch 4q+2t+u
    hh = np.zeros((B, N, HHW), dtype=bf)
    hh[:, :, 0:D] = hb
    hh[:, :, D] = bf(1.0)
    hhq = (hh.reshape(B // 4, 2, 2, N, HHW)
           .transpose(0, 2, 3, 1, 4)
           .reshape(B // 4, 2 * N, 2 * HHW))

    # w_all_q[q, d, (k,l,j)] = hT[q,d,(l,j)] * a[k,d]
    wall = (hTf[:, None, :, :] * a[None, :, :, None]).astype(bf)  # [q,k,d,(l,j)]
    wall = (wall.transpose(0, 2, 1, 3)
            .reshape(B // 4, D, 4 * 4 * N))

    cmba = np.ascontiguousarray(
        np.concatenate([_octify(hT), _octify(wall)], axis=2))
    cmbb = np.ascontiguousarray(
        np.concatenate([_octify(adjTq), _octify(hhq)], axis=2))

    in_maps = []
    for c in range(NCORES):
        gsl = slice(c * OCTS, (c + 1) * OCTS)
        in_maps.append({"cmba": np.ascontiguousarray(cmba[gsl]),
                        "cmbb": np.ascontiguousarray(cmbb[gsl])})
    return in_maps


_NC_CACHE = {}


def run_device(hidden, adj, a, **spmd_kwargs):
    if "nc" not in _NC_CACHE:
        _NC_CACHE["nc"] = build_nc()
    nc = _NC_CACHE["nc"]
    in_maps = prep_inputs(hidden, adj, a)
    res = run_bass_kernel_spmd(nc, in_maps, list(range(NCORES)), **spmd_kwargs)
    # res[c]["out"]: [OCTS, 128, 2*OW]; [g, u*64+i, (gp,t)*HHW + d]
    full = np.concatenate([res.results[c]["out"] for c in range(NCORES)], axis=0)
    full = full.astype(np.float32)
    full = full.reshape(B // 8, 2, N, 2, 2, HHW)             # [g, u, i, gp, t, c]
    num = full[..., 0:D]
    den = full[..., D:D + 1]
    outq = (num / den).transpose(0, 3, 4, 1, 2, 5)           # [g, gp, t, u, i, d]
    out = np.ascontiguousarray(outq.reshape(B, N, D))
    return out.astype(np.float32), res


def kernel(hidden, adj, a):
    out, _ = run_device(hidden, adj, a)
    return out
